# revision 5
# baseline (speedup 1.0000x reference)
"""MoE (63 routed experts, top-7, 1 shared expert) Trainium2 Bass kernel.

Strategy (expert parallelism, per sharding hint):
  - Host: router matmul + softmax + top-k (tiny: 0.7 GFLOP vs 220 GFLOP of
    expert FFNs), token gather per expert.
  - Device (8 NeuronCores, SPMD): each core runs 8 routed-expert slots in
    fp8e4 with DoubleRow matmuls (2 contraction rows/cycle -> ~1.7x the
    fp16 PE rate) plus 1 shared-expert slot in fp16 over a 1/8 token
    slice.  Routed weights are pre-scaled by 256 on the host so they sit
    in the fp8e4 normal range; the 1/256 descale is fused into the GELU
    eviction (scalar engine scale) for layer 1 and folded into the host
    gating for layer 2.  The shared expert stays fp16 because its gate is
    1.0 (fp8 there alone would eat the whole error budget).
  - Host: scatter-add gated expert outputs (+ gate*b2), add shared out,
    bias and residual.

Experts are assigned to slots by descending load rank (rank r -> core r%8,
unit r//8); per-unit capacities are derived from the actual routed loads at
call time (bucket maxima, rounded up to 8), so the NEFF is specialized to
the observed load distribution.  Experts with more than 512 tokens (PSUM
free-dim limit) are split into two virtual experts.  If the virtual expert
count ever exceeds the 64 slots, the excess spills to an exact host-side
FFN (never happens for near-uniform routing).
"""

import numpy as np

B, S, HID = 2, 2048, 1280
E = 63
I = 1280
TOP_K = 7
NCORES = 8
RUNITS = 8         # routed-expert slots per core (fp8)
UNITS = 9          # + 1 shared-expert slot (fp16)
C = 512            # column stride of token tiles (= max capacity)
KO = HID // 128    # 10 contraction chunks
KP = KO // 2       # 5 DoubleRow pair-chunks
T = B * S          # 4096
TSH = T // NCORES  # 512 shared-expert tokens per core

W1CW = 256          # w1 chunk width along I (2 lhsT column groups)
W2CW = 256          # w2 chunk width along H (2 lhsT column groups)
N_W1C = I // W1CW   # 5
N_W2C = HID // W2CW  # 5

WSCALE = 256.0      # routed-weight fp8 pre-scale (power of 2, exact)
WINV = 1.0 / WSCALE

ACT_FUNC = "Gelu"   # simcheck.py overrides to "Identity" (interp lacks Gelu)

_cache = {}


def _build_nc(caps):
    import concourse.mybir as mybir
    import concourse.tile as tile
    from concourse import bacc

    f32 = mybir.dt.float32
    f16 = mybir.dt.float16
    f8 = mybir.dt.float8e4
    GELU = getattr(mybir.ActivationFunctionType, ACT_FUNC)
    DR = mybir.MatmulPerfMode.DoubleRow

    nc = bacc.Bacc(None, target_bir_lowering=False)

    xg_d = nc.dram_tensor("xg", [RUNITS, 128, KO, C], f8, kind="ExternalInput")
    w1_d = nc.dram_tensor("w1", [RUNITS, N_W1C, 128, KO, W1CW], f8,
                          kind="ExternalInput")
    b1_d = nc.dram_tensor("b1", [RUNITS, 128, KO], f32, kind="ExternalInput")
    w2_d = nc.dram_tensor("w2", [RUNITS, N_W2C, 128, KO, W2CW], f8,
                          kind="ExternalInput")
    xsh_d = nc.dram_tensor("xsh", [128, KO, TSH], f16, kind="ExternalInput")
    w1sh_d = nc.dram_tensor("w1sh", [N_W1C, 128, KO, W1CW], f16,
                            kind="ExternalInput")
    b1sh_d = nc.dram_tensor("b1sh", [128, KO], f32, kind="ExternalInput")
    w2sh_d = nc.dram_tensor("w2sh", [N_W2C, 128, KO, W2CW], f16,
                            kind="ExternalInput")
    # transposed output: out[u, p, hk, c] = y[token c, h = hk*128+p]
    # (routed units carry the WSCALE factor; descale folded into host gating)
    out_d = nc.dram_tensor("out", [UNITS, 128, KO, C], f16, kind="ExternalOutput")

    with tile.TileContext(nc) as tc:
        with tc.tile_pool(name="xg_p", bufs=3) as xg_p, \
             tc.tile_pool(name="h1_p", bufs=3) as h1_p, \
             tc.tile_pool(name="w1_p", bufs=4) as w1_p, \
             tc.tile_pool(name="w2_p", bufs=4) as w2_p, \
             tc.tile_pool(name="out_p", bufs=2) as out_p, \
             tc.tile_pool(name="sm_p", bufs=3) as sm_p, \
             tc.tile_pool(name="sh_p", bufs=1) as sh_p, \
             tc.tile_pool(name="wsh_p", bufs=2) as wsh_p, \
             tc.tile_pool(name="ps1_p", bufs=3, space="PSUM") as ps1_p, \
             tc.tile_pool(name="ps2_p", bufs=4, space="PSUM") as ps2_p:

            # ---------------- routed units: fp8 DoubleRow ----------------
            for u in range(RUNITS):
                CAP = caps[u]
                w1cs = {}
                # first w1 chunk ahead of everything else the unit needs
                w1cs[0] = w1_p.tile([128, KO, W1CW], f8, tag="w1c", name="w1c")
                nc.sync.dma_start(w1cs[0][:], w1_d[u, 0])
                xu = xg_p.tile([128, KO, C], f8, tag="xu")
                # split halves so the first matmuls can start sooner
                nc.sync.dma_start(xu[:, :KO // 2], xg_d[u, :, :KO // 2])
                nc.sync.dma_start(xu[:, KO // 2:], xg_d[u, :, KO // 2:])
                b1u = sm_p.tile([128, KO], f32, tag="b1u")
                nc.sync.dma_start(b1u[:], b1_d[u])

                h1 = h1_p.tile([128, KO, C], f8, tag="h1")

                # mm1: h1[i, c] = gelu(WINV * sum_h W1s[h,i] * X^T[h,c] + b1[i])
                for ic in range(N_W1C):
                    if ic not in w1cs:
                        w1cs[ic] = w1_p.tile([128, KO, W1CW], f8, tag="w1c",
                                             name="w1c")
                        nc.sync.dma_start(w1cs[ic][:], w1_d[u, ic])
                    w1c = w1cs[ic]
                    for s in range(W1CW // 128):
                        i_out = ic * (W1CW // 128) + s
                        ps = ps1_p.tile([128, C], f32, tag="ps1")
                        for kp in range(KP):
                            nc.tensor.matmul(
                                ps[:, :CAP],
                                w1c[:, 2 * kp:2 * kp + 2,
                                    s * 128:(s + 1) * 128],
                                xu[:, 2 * kp:2 * kp + 2, :CAP],
                                start=(kp == 0),
                                stop=(kp == KP - 1),
                                perf_mode=DR,
                            )
                        nc.scalar.activation(
                            h1[:, i_out, :CAP], ps[:, :CAP], GELU,
                            bias=b1u[:, i_out:i_out + 1], scale=WINV)

                # mm2 (transposed): yT[h, c] = sum_i W2s[i, h] * h1[i, c]
                oy = out_p.tile([128, KO, C], f16, tag="oy")
                for hcc in range(N_W2C):
                    w2c = w2_p.tile([128, KO, W2CW], f8, tag="w2c")
                    nc.sync.dma_start(w2c[:], w2_d[u, hcc])
                    for s2 in range(W2CW // 128):
                        hk = hcc * (W2CW // 128) + s2
                        ps2 = ps2_p.tile([128, C], f32, tag="ps2")
                        for kp in range(KP):
                            nc.tensor.matmul(
                                ps2[:, :CAP],
                                w2c[:, 2 * kp:2 * kp + 2,
                                    s2 * 128:(s2 + 1) * 128],
                                h1[:, 2 * kp:2 * kp + 2, :CAP],
                                start=(kp == 0),
                                stop=(kp == KP - 1),
                                perf_mode=DR,
                            )
                        nc.vector.tensor_copy(oy[:, hk, :CAP], ps2[:, :CAP])
                        # drain finished output rows early so the final DMA
                        # (and the kernel tail) stays small
                        if hk % 2 == 1:
                            nc.sync.dma_start(
                                out_d[u, :, hk - 1:hk + 1, :CAP],
                                oy[:, hk - 1:hk + 1, :CAP])

            # ---------------- shared unit: fp16 ----------------
            w1shcs = {}
            w1shcs[0] = wsh_p.tile([128, KO, W1CW], f16, tag="wshc", name="wshc")
            nc.sync.dma_start(w1shcs[0][:], w1sh_d[0])
            xsh = sh_p.tile([128, KO, TSH], f16, tag="xsh")
            nc.sync.dma_start(xsh[:], xsh_d[:])
            b1s = sm_p.tile([128, KO], f32, tag="b1u")
            nc.sync.dma_start(b1s[:], b1sh_d[:])

            h1s = sh_p.tile([128, KO, TSH], f16, tag="h1sh")
            for ic in range(N_W1C):
                if ic not in w1shcs:
                    w1shcs[ic] = wsh_p.tile([128, KO, W1CW], f16, tag="wshc",
                                            name="wshc")
                    nc.sync.dma_start(w1shcs[ic][:], w1sh_d[ic])
                w1c = w1shcs[ic]
                for s in range(W1CW // 128):
                    i_out = ic * (W1CW // 128) + s
                    ps = ps1_p.tile([128, C], f32, tag="ps1")
                    for ko in range(KO):
                        nc.tensor.matmul(
                            ps[:, :TSH],
                            w1c[:, ko, s * 128:(s + 1) * 128],
                            xsh[:, ko],
                            start=(ko == 0),
                            stop=(ko == KO - 1),
                        )
                    nc.scalar.activation(
                        h1s[:, i_out], ps[:, :TSH], GELU,
                        bias=b1s[:, i_out:i_out + 1])

            oys = sh_p.tile([128, KO, TSH], f16, tag="oysh")
            for hcc in range(N_W2C):
                w2c = wsh_p.tile([128, KO, W2CW], f16, tag="w2shc")
                nc.sync.dma_start(w2c[:], w2sh_d[hcc])
                for s2 in range(W2CW // 128):
                    hk = hcc * (W2CW // 128) + s2
                    ps2 = ps2_p.tile([128, C], f32, tag="ps2")
                    for ko in range(KO):
                        nc.tensor.matmul(
                            ps2[:, :TSH],
                            w2c[:, ko, s2 * 128:(s2 + 1) * 128],
                            h1s[:, ko],
                            start=(ko == 0),
                            stop=(ko == KO - 1),
                        )
                    nc.vector.tensor_copy(oys[:, hk], ps2[:, :TSH])
                    if hk % 2 == 1:
                        nc.sync.dma_start(
                            out_d[8, :, hk - 1:hk + 1, :TSH],
                            oys[:, hk - 1:hk + 1])

    nc.compile()
    return nc


def _get_nc(caps):
    key = tuple(caps)
    if key not in _cache:
        _cache[key] = _build_nc(key)
    return _cache[key]


def _f8_dt():
    import ml_dtypes
    return np.dtype(ml_dtypes.float8_e4m3)


def _gelu_np(v):
    from scipy.special import erf
    v = v.astype(np.float32)
    return (0.5 * v * (1.0 + erf(v / np.sqrt(2.0)))).astype(np.float32)


def _tile_w1(w):
    # [H, I] -> [N_W1C, 128, KO, W1CW] with w1t[ic, p, ko, j] = w[ko*128+p, ic*W1CW+j]
    return w.reshape(KO, 128, N_W1C, W1CW).transpose(2, 1, 0, 3)


def _tile_w2(w):
    # [I, H] -> [N_W2C, 128, KO, W2CW]
    return w.reshape(KO, 128, N_W2C, W2CW).transpose(2, 1, 0, 3)


def _ensure_axon_hooks_stub():
    """bass_utils' axon trace path imports antenv.axon_hooks, which this
    image lacks; provide a no-op stub so a BASS_TRACE-enabled environment
    degrades gracefully instead of crashing."""
    import sys
    import types
    try:
        import antenv.axon_hooks  # noqa: F401
        return
    except ImportError:
        pass
    try:
        import antenv
    except ImportError:
        return
    mod = types.ModuleType("antenv.axon_hooks")
    holder = [None]
    mod.set_axon_ntff_profile_hook = lambda h: holder.__setitem__(0, h)
    mod.get_axon_ntff_profile_hook = lambda: holder[0]
    sys.modules["antenv.axon_hooks"] = mod
    antenv.axon_hooks = mod


def kernel(x, w1_shared, b1_shared, w2_shared, b2_shared,
           router_w, router_b, w1, b1, w2, b2):
    _ensure_axon_hooks_stub()
    from concourse.bass_utils import run_bass_kernel_spmd

    f8 = _f8_dt()

    x = np.asarray(x, np.float32)
    w1 = np.asarray(w1, np.float32)
    b1 = np.asarray(b1, np.float32)
    w2 = np.asarray(w2, np.float32)
    b2 = np.asarray(b2, np.float32)
    w1_shared = np.asarray(w1_shared, np.float32)
    b1_shared = np.asarray(b1_shared, np.float32)
    w2_shared = np.asarray(w2_shared, np.float32)
    b2_shared = np.asarray(b2_shared, np.float32)
    router_w = np.asarray(router_w, np.float32)
    router_b = np.asarray(router_b, np.float32)

    xf = x.reshape(T, HID)

    # ---------------- host routing ----------------
    logits = xf @ router_w + router_b
    m = logits.max(-1, keepdims=True)
    ex = np.exp(logits - m, dtype=np.float32)
    affin = ex / ex.sum(-1, keepdims=True, dtype=np.float32)
    order = np.argsort(-affin, axis=-1, kind="stable")[:, :TOP_K]   # [T, K]
    vals = np.take_along_axis(affin, order, axis=-1)                # [T, K]

    # group (token, gate) pairs by expert
    flat_e = order.ravel()
    flat_t = np.repeat(np.arange(T), TOP_K)
    flat_g = vals.ravel()
    sort = np.argsort(flat_e, kind="stable")
    se, st, sg = flat_e[sort], flat_t[sort], flat_g[sort]
    starts = np.searchsorted(se, np.arange(E + 1))

    # virtual experts: split any load > C into two halves (PSUM free-dim cap)
    virt = []   # (expert, tokens, gates)
    for e in range(E):
        toks = st[starts[e]:starts[e + 1]]
        gs = sg[starts[e]:starts[e + 1]]
        while len(toks) > C:
            virt.append((e, toks[:C // 2], gs[:C // 2]))
            toks, gs = toks[C // 2:], gs[C // 2:]
        virt.append((e, toks, gs))

    # slot table: 64 slots; rank r -> core r%8, unit r//8.  Per-unit caps =
    # bucket maxima of the descending load distribution, rounded up to 8.
    NSLOT = NCORES * RUNITS
    virt.sort(key=lambda t: -len(t[1]))
    host_fallback = virt[NSLOT:]
    virt = virt[:NSLOT]
    slot_expert = [-1] * NSLOT
    slot_tok = [np.empty(0, np.int64)] * NSLOT
    slot_gate = [np.empty(0, np.float32)] * NSLOT
    caps = [8] * RUNITS
    for r, (e, toks, gs) in enumerate(virt):
        s = (r % NCORES) * RUNITS + (r // NCORES)
        slot_expert[s] = e
        slot_tok[s] = toks
        slot_gate[s] = gs
        u = r // NCORES
        caps[u] = max(caps[u], (len(toks) + 7) // 8 * 8)

    # ---------------- build per-core device inputs ----------------
    # x transposed + partition-tiled: xT_t[ko, p, t] = x[t, ko*128+p]
    xT = np.ascontiguousarray(xf.T).reshape(KO, 128, T)
    xT8 = xT.astype(f8)
    xT16 = xT.astype(np.float16)

    w1t_sh = _tile_w1(w1_shared[0]).astype(np.float16)
    w2t_sh = _tile_w2(w2_shared[0]).astype(np.float16)
    b1t_sh = np.ascontiguousarray(b1_shared[0].reshape(KO, 128).T)

    in_maps = []
    for c in range(NCORES):
        xg = np.zeros((RUNITS, 128, KO, C), f8)
        w1u = np.zeros((RUNITS, N_W1C, 128, KO, W1CW), f8)
        b1u = np.zeros((RUNITS, 128, KO), np.float32)
        w2u = np.zeros((RUNITS, N_W2C, 128, KO, W2CW), f8)
        for u in range(RUNITS):
            s = c * RUNITS + u
            e = slot_expert[s]
            if e < 0 or len(slot_tok[s]) == 0:
                continue
            n = len(slot_tok[s])
            idx = np.zeros(C, np.int64)
            idx[:n] = slot_tok[s]
            xg[u] = xT8[:, :, idx].swapaxes(0, 1)
            w1u[u] = _tile_w1(w1[e] * WSCALE).astype(f8)
            b1u[u] = b1[e].reshape(KO, 128).T
            w2u[u] = _tile_w2(w2[e] * WSCALE).astype(f8)
        in_maps.append({
            "xg": xg, "w1": w1u, "b1": b1u, "w2": w2u,
            "xsh": np.ascontiguousarray(
                xT16[:, :, c * TSH:(c + 1) * TSH].swapaxes(0, 1)),
            "w1sh": w1t_sh, "b1sh": b1t_sh, "w2sh": w2t_sh,
        })

    # ---------------- run on 8 cores ----------------
    nc = _get_nc(caps)
    res = run_bass_kernel_spmd(nc, in_maps, core_ids=list(range(NCORES)))
    outs = [r["out"] for r in res.results]   # [UNITS, 128, KO, C] f16 each

    # ---------------- host unshard / scatter ----------------
    # device output is transposed: outs[c][u][p, hk, c'] = y[c', hk*128+p]
    def untile_y(o, n):
        return o.transpose(1, 0, 2).reshape(HID, C)[:, :n].T.astype(np.float32)

    acc = np.zeros((T, HID), np.float32)     # shared + routed
    # shared expert (unit 8 on each core), gate 1, + b2_shared
    for c in range(NCORES):
        ys = untile_y(outs[c][8][:, :, :TSH].astype(np.float32)
                      if C != TSH else outs[c][8], TSH)
        acc[c * TSH:(c + 1) * TSH] = ys + b2_shared[0]
    # routed experts: gate * (y/WSCALE + b2), scattered by token
    for s in range(NSLOT):
        e = slot_expert[s]
        n = len(slot_tok[s])
        if e < 0 or n == 0:
            continue
        ye = untile_y(outs[s // RUNITS][s % RUNITS], n)
        # token indices are unique within one slot, so fancy += is safe
        acc[slot_tok[s]] += slot_gate[s][:, None] * (ye * WINV + b2[e][None, :])
    # exact host fallback if virtual experts ever exceed the 64 slots
    for e, toks, gs in host_fallback:
        if len(toks) == 0:
            continue
        h = _gelu_np(xf[toks] @ w1[e] + b1[e])
        acc[toks] += gs[:, None] * (h @ w2[e] + b2[e])

    return (acc + xf).reshape(B, S, HID).astype(np.float32)
